# revision 6
# baseline (speedup 1.0000x reference)
# Trainium2 Bass kernel for nn_CustomGate: y = (I_L (x) M (x) I_R) @ x
# with D=2, N=13, INDEX=5 -> L=32, R=128, DIM=8192, BATCH=2048, complex64.
#
# Math: viewing x as [L, D, R, B], the gate mixes only the D axis:
#   y[l, a, r, b] = sum_b' M[a, b'] x[l, b', r, b]
# Splitting complex into real/imag gives, per (l, r, b), a fixed real 4x4
# mix A = [[Mr, -Mi], [Mi, Mr]] over components (x0r, x1r, x0i, x1i).
#
# Sharding: L axis across 8 cores -> core i owns rows [1024*i, 1024*(i+1))
# of x_real/x_imag (contiguous slabs, no cross-core communication).
#
# The host pre-interleaves each core's slab into xcat [128, 4*8192] fp32:
# partition p = comp*32 + q (comp in {x0r, x1r, x0i, x1i}, q = r_hi) and
# free = l*8192 + rl*2048 + b (r = q*4 + rl). Device DMAs are then fully
# contiguous [128, 32KB] slabs. One fp32 TensorE matmul per 512-col chunk
# against the stationary W = A^T (x) I_32 (host-precomputed, [128, 128])
# produces all 4 output components in one pass. PSUM is evicted to SBUF
# (DVE/ACT alternating) and DMA'd out contiguously (separate HWDGE ring
# from the input DMAs), then the host de-interleaves.

import numpy as np

N_CORES = 8
DIM = 8192
BATCH = 2048
ROWS_PER_CORE = DIM // N_CORES  # 1024
NL = ROWS_PER_CORE // 256  # 4 l-blocks per core
FREE = 4 * BATCH  # 8192 free elements per l-block
JCH = 512  # matmul free-dim chunk (one PSUM bank of fp32)
CH = 4096  # pipeline chunk (free elements; 2 MB per [128, CH] f32 tile)
NCH = NL * FREE // CH  # 8 chunks per core
NJ = CH // JCH  # 8 matmuls per chunk

_PROGRAM = None


def _build_program():
    import concourse.bacc as bacc
    import concourse.tile as tile
    import concourse.mybir as mybir

    F32 = mybir.dt.float32

    # Bacc (not raw Bass): its compile() runs move_matmul_waits_to_ldweights
    # + generate_event_semaphores, which legalize multi-wait instructions for
    # TRN2 (at most 1 sync wait per instruction).
    nc = bacc.Bacc("TRN2", target_bir_lowering=False)
    w = nc.declare_dram_parameter("w", [128, 128], F32, isOutput=False)
    xin = nc.declare_dram_parameter("xin", [128, NL * FREE], F32, isOutput=False)
    yout = nc.declare_dram_parameter("yout", [128, NL * FREE], F32, isOutput=True)

    with tile.TileContext(nc) as tc:
        with (
            tc.tile_pool(name="wpool", bufs=1) as wpool,
            tc.tile_pool(name="inpool", bufs=3) as inpool,
            tc.tile_pool(name="outpool", bufs=2) as outpool,
            tc.tile_pool(name="psum", bufs=8, space="PSUM") as psumpool,
        ):
            wt = wpool.tile([128, 128], F32)
            nc.sync.dma_start(out=wt[:], in_=w[:])
            for c in range(NCH):
                xt = inpool.tile([128, CH], F32, tag="xt")
                nc.sync.dma_start(out=xt[:], in_=xin[:, c * CH : (c + 1) * CH])
                yt = outpool.tile([128, CH], F32, tag="yt")
                for j in range(NJ):
                    ps = psumpool.tile([128, JCH], F32)
                    nc.tensor.matmul(
                        ps[:],
                        lhsT=wt[:],
                        rhs=xt[:, j * JCH : (j + 1) * JCH],
                        start=True,
                        stop=True,
                    )
                    if j % 2 == 0:
                        nc.vector.tensor_copy(yt[:, j * JCH : (j + 1) * JCH], ps[:])
                    else:
                        nc.scalar.copy(yt[:, j * JCH : (j + 1) * JCH], ps[:])
                # output on the ACT HWDGE ring so input/output DMAs round-robin
                # on the SDMA engines instead of queuing FIFO behind each other
                nc.scalar.dma_start(out=yout[:, c * CH : (c + 1) * CH], in_=yt[:])
    nc.compile()
    return nc


def _get_program():
    global _PROGRAM
    if _PROGRAM is None:
        _PROGRAM = _build_program()
    return _PROGRAM


def _make_w(M_real, M_imag):
    Mr = np.asarray(M_real, dtype=np.float32)
    Mi = np.asarray(M_imag, dtype=np.float32)
    # components in = (x0r, x1r, x0i, x1i), out = (y0r, y1r, y0i, y1i)
    A = np.block([[Mr, -Mi], [Mi, Mr]]).astype(np.float32)  # [4, 4]
    # matmul computes out[i, j] = sum_k W[k, i] rhs[k, j]; k/i = (comp, q)
    W = np.kron(A.T, np.eye(32, dtype=np.float32)).astype(np.float32)
    return np.ascontiguousarray(W)


def _interleave(slab):
    # [1024, 2048] -> [64, 4*8192]: [l, d, q, rl, b] -> [(d q), (l rl b)]
    xs = slab.reshape(NL, 2, 32, 4, BATCH)
    return xs.transpose(1, 2, 0, 3, 4).reshape(64, NL * FREE)


def _deinterleave(half):
    # [64, 4*8192] -> [1024, 2048]
    ys = half.reshape(2, 32, NL, 4, BATCH)
    return ys.transpose(2, 0, 1, 3, 4).reshape(ROWS_PER_CORE, BATCH)


def _in_maps(W, x_real, x_imag):
    maps = []
    for i in range(N_CORES):
        sl = slice(i * ROWS_PER_CORE, (i + 1) * ROWS_PER_CORE)
        xcat = np.empty((128, NL * FREE), dtype=np.float32)
        xcat[0:64] = _interleave(x_real[sl])
        xcat[64:128] = _interleave(x_imag[sl])
        maps.append({"w": W, "xin": xcat})
    return maps


def _gather(results):
    y = np.empty((DIM, BATCH), dtype=np.complex64)
    for i in range(N_CORES):
        sl = slice(i * ROWS_PER_CORE, (i + 1) * ROWS_PER_CORE)
        ycat = results[i]["yout"]
        y.real[sl] = _deinterleave(ycat[0:64])
        y.imag[sl] = _deinterleave(ycat[64:128])
    return y


def kernel(M_real, M_imag, x_real, x_imag):
    from concourse import bass_utils

    x_real = np.asarray(x_real, dtype=np.float32)
    x_imag = np.asarray(x_imag, dtype=np.float32)
    W = _make_w(M_real, M_imag)

    nc = _get_program()
    res = bass_utils.run_bass_kernel_spmd(
        nc, _in_maps(W, x_real, x_imag), list(range(N_CORES))
    )
    return _gather(res.results)


# revision 10
# speedup vs baseline: 1.0171x; 1.0171x over previous
# Trainium2 Bass kernel for nn_CustomGate: y = (I_L (x) M (x) I_R) @ x
# with D=2, N=13, INDEX=5 -> L=32, R=128, DIM=8192, BATCH=2048, complex64.
#
# Math: viewing x as [L, D, R, B], the gate mixes only the D axis:
#   y[l, a, r, b] = sum_b' M[a, b'] x[l, b', r, b]
# Splitting complex into real/imag gives, per (l, r, b), a fixed real 4x4
# mix A = [[Mr, -Mi], [Mi, Mr]] over components (x0r, x1r, x0i, x1i).
#
# Sharding: L axis across 8 cores -> core i owns rows [1024*i, 1024*(i+1))
# of x_real/x_imag (contiguous slabs, no cross-core communication).
#
# The host pre-interleaves each core's slab into xcat [128, 4*8192] fp32:
# partition p = comp*32 + q (comp in {x0r, x1r, x0i, x1i}, q = r_hi) and
# free = l*8192 + rl*2048 + b (r = q*4 + rl). Device DMAs are then fully
# contiguous [128, 32KB] slabs. One fp32 TensorE matmul per 512-col chunk
# against the stationary W = A^T (x) I_32 (host-precomputed, [128, 128])
# produces all 4 output components in one pass. PSUM is evicted to SBUF
# (DVE/ACT alternating) and DMA'd out contiguously (separate HWDGE ring
# from the input DMAs), then the host de-interleaves.

import numpy as np

N_CORES = 8
DIM = 8192
BATCH = 2048
ROWS_PER_CORE = DIM // N_CORES  # 1024
NL = ROWS_PER_CORE // 256  # 4 l-blocks per core
FREE = 4 * BATCH  # 8192 free elements per l-block
JCH = 512  # matmul free-dim chunk (one PSUM bank of fp32)
# Tapered pipeline chunks (free elements; 512 free = 256 KB tile):
# small chunks at the start (matmuls begin sooner) and at the end (the final
# in->matmul->evict->out serial chain is short); big 4 MB chunks keep DMA
# efficiency in steady state. Sum = NL*FREE = 32768.
CHUNKS = [2048, 2048, 4096, 8192, 8192, 4096, 2048, 2048]
FP32R = False  # fp32r needs rounded producers (reduced precision); keep fp32

_PROGRAM = None


def _build_program():
    import concourse.bacc as bacc
    import concourse.tile as tile
    import concourse.mybir as mybir

    F32 = mybir.dt.float32

    # Bacc (not raw Bass): its compile() runs move_matmul_waits_to_ldweights
    # + generate_event_semaphores, which legalize multi-wait instructions for
    # TRN2 (at most 1 sync wait per instruction).
    nc = bacc.Bacc("TRN2", target_bir_lowering=False)
    w = nc.declare_dram_parameter("w", [128, 128], F32, isOutput=False)
    xin = nc.declare_dram_parameter("xin", [128, NL * FREE], F32, isOutput=False)
    yout = nc.declare_dram_parameter("yout", [128, NL * FREE], F32, isOutput=True)

    with tile.TileContext(nc) as tc:
        with (
            tc.tile_pool(name="wpool", bufs=1) as wpool,
            tc.tile_pool(name="inpool", bufs=3) as inpool,
            tc.tile_pool(name="outpool", bufs=2) as outpool,
            tc.tile_pool(name="psum", bufs=8, space="PSUM") as psumpool,
        ):
            wt = wpool.tile([128, 128], F32)
            nc.sync.dma_start(out=wt[:], in_=w[:])
            off = 0
            for ch in CHUNKS:
                xt = inpool.tile([128, ch], F32, tag="xt")
                nc.sync.dma_start(out=xt[:], in_=xin[:, off : off + ch])
                yt = outpool.tile([128, ch], F32, tag="yt")
                for j in range(ch // JCH):
                    ps = psumpool.tile([128, JCH], F32)
                    nc.tensor.matmul(
                        ps[:],
                        lhsT=wt[:],
                        rhs=xt[:, j * JCH : (j + 1) * JCH],
                        start=True,
                        stop=True,
                    )
                    if j % 2 == 0:
                        nc.vector.tensor_copy(yt[:, j * JCH : (j + 1) * JCH], ps[:])
                    else:
                        nc.scalar.copy(yt[:, j * JCH : (j + 1) * JCH], ps[:])
                # output on the ACT HWDGE ring so input/output DMAs round-robin
                # on the SDMA engines instead of queuing FIFO behind each other
                nc.scalar.dma_start(out=yout[:, off : off + ch], in_=yt[:])
                off += ch
    nc.compile()
    return nc


def _get_program():
    global _PROGRAM
    if _PROGRAM is None:
        _PROGRAM = _build_program()
    return _PROGRAM


def _make_w(M_real, M_imag):
    Mr = np.asarray(M_real, dtype=np.float32)
    Mi = np.asarray(M_imag, dtype=np.float32)
    # components in = (x0r, x1r, x0i, x1i), out = (y0r, y1r, y0i, y1i)
    A = np.block([[Mr, -Mi], [Mi, Mr]]).astype(np.float32)  # [4, 4]
    # matmul computes out[i, j] = sum_k W[k, i] rhs[k, j]; k/i = (comp, q)
    W = np.kron(A.T, np.eye(32, dtype=np.float32)).astype(np.float32)
    return np.ascontiguousarray(W)


def _interleave(slab):
    # [1024, 2048] -> [64, 4*8192]: [l, d, q, rl, b] -> [(d q), (l rl b)]
    xs = slab.reshape(NL, 2, 32, 4, BATCH)
    return xs.transpose(1, 2, 0, 3, 4).reshape(64, NL * FREE)


def _deinterleave(half):
    # [64, 4*8192] -> [1024, 2048]
    ys = half.reshape(2, 32, NL, 4, BATCH)
    return ys.transpose(2, 0, 1, 3, 4).reshape(ROWS_PER_CORE, BATCH)


def _in_maps(W, x_real, x_imag):
    maps = []
    for i in range(N_CORES):
        sl = slice(i * ROWS_PER_CORE, (i + 1) * ROWS_PER_CORE)
        xcat = np.empty((128, NL * FREE), dtype=np.float32)
        xcat[0:64] = _interleave(x_real[sl])
        xcat[64:128] = _interleave(x_imag[sl])
        maps.append({"w": W, "xin": xcat})
    return maps


def _gather(results):
    y = np.empty((DIM, BATCH), dtype=np.complex64)
    for i in range(N_CORES):
        sl = slice(i * ROWS_PER_CORE, (i + 1) * ROWS_PER_CORE)
        ycat = results[i]["yout"]
        y.real[sl] = _deinterleave(ycat[0:64])
        y.imag[sl] = _deinterleave(ycat[64:128])
    return y


def kernel(M_real, M_imag, x_real, x_imag):
    from concourse import bass_utils

    x_real = np.asarray(x_real, dtype=np.float32)
    x_imag = np.asarray(x_imag, dtype=np.float32)
    W = _make_w(M_real, M_imag)

    nc = _get_program()
    res = bass_utils.run_bass_kernel_spmd(
        nc, _in_maps(W, x_real, x_imag), list(range(N_CORES))
    )
    return _gather(res.results)


# revision 11
# speedup vs baseline: 1.1785x; 1.1587x over previous
# Trainium2 Bass kernel for nn_CustomGate: y = (I_L (x) M (x) I_R) @ x
# with D=2, N=13, INDEX=5 -> L=32, R=128, DIM=8192, BATCH=2048, complex64.
#
# Math: viewing x as [L, D, R, B], the gate mixes only the D axis:
#   y[l, a, r, b] = sum_b' M[a, b'] x[l, b', r, b]
# Splitting complex into real/imag gives, per (l, r, b), a fixed real 4x4
# mix A = [[Mr, -Mi], [Mi, Mr]] over components (x0r, x1r, x0i, x1i).
#
# Sharding: L axis across 8 cores -> core i owns rows [1024*i, 1024*(i+1))
# of x_real/x_imag (contiguous slabs, no cross-core communication).
#
# The host pre-interleaves each core's slab into xcat [128, 4*8192] fp32:
# partition p = comp*32 + q (comp in {x0r, x1r, x0i, x1i}, q = r_hi) and
# free = l*8192 + rl*2048 + b (r = q*4 + rl). Device DMAs are then fully
# contiguous [128, 32KB] slabs. One fp32 TensorE matmul per 512-col chunk
# against the stationary W = A^T (x) I_32 (host-precomputed, [128, 128])
# produces all 4 output components in one pass. PSUM is evicted to SBUF
# (DVE/ACT alternating) and DMA'd out contiguously (separate HWDGE ring
# from the input DMAs), then the host de-interleaves.

import numpy as np

N_CORES = 8
DIM = 8192
BATCH = 2048
ROWS_PER_CORE = DIM // N_CORES  # 1024
NL = ROWS_PER_CORE // 256  # 4 l-blocks per core
FREE = 4 * BATCH  # 8192 free elements per l-block
JCH = 512  # matmul free-dim chunk (one PSUM bank of fp32)
# Tapered pipeline chunks (free elements; 512 free = 256 KB tile):
# small chunks at the start (matmuls begin sooner) and at the end (the final
# in->matmul->evict->out serial chain is short); big 4 MB chunks keep DMA
# efficiency in steady state. Sum = NL*FREE = 32768.
CHUNKS = [4096, 8192, 8192, 8192, 2048, 2048]
FP32R = False  # fp32r needs rounded producers (reduced precision); keep fp32

_PROGRAM = None


def _build_program():
    import concourse.bacc as bacc
    import concourse.tile as tile
    import concourse.mybir as mybir

    F32 = mybir.dt.float32

    # Bacc (not raw Bass): its compile() runs move_matmul_waits_to_ldweights
    # + generate_event_semaphores, which legalize multi-wait instructions for
    # TRN2 (at most 1 sync wait per instruction).
    nc = bacc.Bacc("TRN2", target_bir_lowering=False)
    w = nc.declare_dram_parameter("w", [128, 128], F32, isOutput=False)
    xin = nc.declare_dram_parameter("xin", [128, NL * FREE], F32, isOutput=False)
    yout = nc.declare_dram_parameter("yout", [128, NL * FREE], F32, isOutput=True)

    with tile.TileContext(nc) as tc:
        with (
            tc.tile_pool(name="wpool", bufs=1) as wpool,
            tc.tile_pool(name="inpool", bufs=3) as inpool,
            tc.tile_pool(name="outpool", bufs=2) as outpool,
            tc.tile_pool(name="psum", bufs=8, space="PSUM") as psumpool,
        ):
            wt = wpool.tile([128, 128], F32)
            nc.sync.dma_start(out=wt[:], in_=w[:])
            off = 0
            for ch in CHUNKS:
                xt = inpool.tile([128, ch], F32, tag="xt")
                nc.sync.dma_start(out=xt[:], in_=xin[:, off : off + ch])
                yt = outpool.tile([128, ch], F32, tag="yt")
                for j in range(ch // JCH):
                    ps = psumpool.tile([128, JCH], F32)
                    nc.tensor.matmul(
                        ps[:],
                        lhsT=wt[:],
                        rhs=xt[:, j * JCH : (j + 1) * JCH],
                        start=True,
                        stop=True,
                    )
                    if j % 2 == 0:
                        nc.vector.tensor_copy(yt[:, j * JCH : (j + 1) * JCH], ps[:])
                    else:
                        nc.scalar.copy(yt[:, j * JCH : (j + 1) * JCH], ps[:])
                # output on the ACT HWDGE ring so input/output DMAs round-robin
                # on the SDMA engines instead of queuing FIFO behind each other
                nc.scalar.dma_start(out=yout[:, off : off + ch], in_=yt[:])
                off += ch
    nc.compile()
    return nc


def _get_program():
    global _PROGRAM
    if _PROGRAM is None:
        _PROGRAM = _build_program()
    return _PROGRAM


def _make_w(M_real, M_imag):
    Mr = np.asarray(M_real, dtype=np.float32)
    Mi = np.asarray(M_imag, dtype=np.float32)
    # components in = (x0r, x1r, x0i, x1i), out = (y0r, y1r, y0i, y1i)
    A = np.block([[Mr, -Mi], [Mi, Mr]]).astype(np.float32)  # [4, 4]
    # matmul computes out[i, j] = sum_k W[k, i] rhs[k, j]; k/i = (comp, q)
    W = np.kron(A.T, np.eye(32, dtype=np.float32)).astype(np.float32)
    return np.ascontiguousarray(W)


def _interleave(slab):
    # [1024, 2048] -> [64, 4*8192]: [l, d, q, rl, b] -> [(d q), (l rl b)]
    xs = slab.reshape(NL, 2, 32, 4, BATCH)
    return xs.transpose(1, 2, 0, 3, 4).reshape(64, NL * FREE)


def _deinterleave(half):
    # [64, 4*8192] -> [1024, 2048]
    ys = half.reshape(2, 32, NL, 4, BATCH)
    return ys.transpose(2, 0, 1, 3, 4).reshape(ROWS_PER_CORE, BATCH)


def _in_maps(W, x_real, x_imag):
    maps = []
    for i in range(N_CORES):
        sl = slice(i * ROWS_PER_CORE, (i + 1) * ROWS_PER_CORE)
        xcat = np.empty((128, NL * FREE), dtype=np.float32)
        xcat[0:64] = _interleave(x_real[sl])
        xcat[64:128] = _interleave(x_imag[sl])
        maps.append({"w": W, "xin": xcat})
    return maps


def _gather(results):
    y = np.empty((DIM, BATCH), dtype=np.complex64)
    for i in range(N_CORES):
        sl = slice(i * ROWS_PER_CORE, (i + 1) * ROWS_PER_CORE)
        ycat = results[i]["yout"]
        y.real[sl] = _deinterleave(ycat[0:64])
        y.imag[sl] = _deinterleave(ycat[64:128])
    return y


def kernel(M_real, M_imag, x_real, x_imag):
    from concourse import bass_utils

    x_real = np.asarray(x_real, dtype=np.float32)
    x_imag = np.asarray(x_imag, dtype=np.float32)
    W = _make_w(M_real, M_imag)

    nc = _get_program()
    res = bass_utils.run_bass_kernel_spmd(
        nc, _in_maps(W, x_real, x_imag), list(range(N_CORES))
    )
    return _gather(res.results)
